# revision 1
# baseline (speedup 1.0000x reference)
"""Causal single-head attention (b=4, s=2048, d=1024, h=64) on 8 TRN2 cores.

Sharding: core c -> (batch b = c//2, g = c%2), where the core owns the
balanced q-chunk pair A,B = (0,3) if g==0 else (1,2) (512 queries each) --
both cores of a batch do 20 useful 128-wide k-blocks of causal work.

Each core receives x[b] host-transposed to [d, s] with rows permuted to
[A; B; rest0; rest1], which makes one uniform SPMD program valid for all
8 cores:

  - K/V are projected for all 2048 permuted rows ([Wv|Wk] packed, M=128),
    Q (pre-scaled by 1/sqrt(h) on the host) only for the first 1024 rows,
    via tile_position=(0,64) so Q lands on PSUM partitions 64:128 and no
    cross-partition copies are ever needed.
  - Scores are computed transposed, sT[k, q], contraction over h on
    partitions 64:128, so the softmax sum over k and attn@V are both
    TensorE matmuls over the partition axis.
  - exp() runs without max-subtraction: scores are ~N(0, 0.33^2) for this
    problem's input distribution, so exp is in [~0.1, ~10] -- safe.
  - The causal triangle of the in-chunk diagonal blocks is applied by
    multiplying exp-scores with 4 gpsimd-generated [128, 512] mask tiles.
  - Cross-chunk blocks are gated per (core, slot, rest-chunk) by an
    additive exp bias in {0, -40} from the packed bt input (exp(-40)~0).
  - V gets an appended ones-column, so the AV matmul emits
    [o_unnorm; denom] in one pass.  The host divides and reassembles.

All matmul operands are bf16 (full PE speed, separate-LDWEIGHTS path);
accumulation is fp32 in PSUM.  Host casts x/W to bf16, halving the x DMA.
Requires bacc.Bacc + nc.compile() (event-semaphore lowering of
multi-waits; raw bass.Bass fails walrus codegen on this stack).
"""

import numpy as np

B, S, D, H = 4, 2048, 1024, 64
P = 128
HALF = S // 2          # 1024 queries per core
CH = 512               # free-dim chunk (PSUM bank = 512 fp32)
KSUB = D // P          # 8 contraction subtiles for projections
NCHUNK = S // CH       # 4 sequence chunks
NBLK = CH // P         # 4 k-blocks per chunk

_NC = None
TRACE = False
LAST = {}


def build_bass():
    import concourse.bass as bass  # noqa: F401
    import concourse.mybir as mybir
    import concourse.tile as tile
    from concourse import bacc
    from concourse.masks import make_identity

    f32 = mybir.dt.float32
    bf16 = mybir.dt.bfloat16
    AF = mybir.ActivationFunctionType

    nc = bacc.Bacc()
    xt_d = nc.dram_tensor("xt", [D, S], bf16, kind="ExternalInput")
    # [Wv | Wk | Wq/8] packed in one tensor; biases+theta packed in another,
    # so the pre-chunk-1 DMA count (and its per-DMA overhead) is minimal.
    # Host pre-swizzles weights to [P, KSUB*(P+H)] (partition-major) so the
    # DMA lines are 2KB contiguous -- [D, 192] row-major gave 256B lines,
    # which pay a 2x DMA latency penalty (<512B threshold).
    wall_d = nc.dram_tensor("wall", [P, KSUB * (P + H)], bf16, kind="ExternalInput")
    bt_d = nc.dram_tensor("bt", [P, 6], f32, kind="ExternalInput")
    out_d = nc.dram_tensor("out", [H + 1, HALF], f32, kind="ExternalOutput")

    xt_r = xt_d.rearrange("(o p) s -> p o s", p=P)
    wvk_r = wall_d[:, :KSUB * P].rearrange("p (o m) -> p o m", o=KSUB)
    wq_r = wall_d[:, KSUB * P:].rearrange("p (o m) -> p o m", o=KSUB)

    with tile.TileContext(nc) as tc:
        with (
            tc.tile_pool(name="consts", bufs=1) as consts,
            tc.tile_pool(name="data", bufs=1) as data,
            tc.tile_pool(name="ps", bufs=5, space="PSUM") as pspool,
            tc.tile_pool(name="pstp", bufs=1, space="PSUM") as pstpool,
            tc.tile_pool(name="po", bufs=1, space="PSUM") as popool,
            tc.tile_pool(name="pt", bufs=6) as ptpool,
        ):
            wvk = consts.tile([P, KSUB, P], bf16)
            wq = consts.tile([P, KSUB, H], bf16)
            bt = consts.tile([P, 6], f32)
            maskt = consts.tile([P, NBLK, CH], bf16)
            ident = consts.tile([H, H], bf16)
            biases = bt[:, 0:2]
            theta = bt[:, 2:6]
            nc.sync.dma_start(wvk[:], wvk_r[:])
            make_identity(nc, ident[:])

            xt = [[data.tile([P, CH], bf16, tag=f"xt{c}_{o}", name=f"xt{c}_{o}")
                   for o in range(KSUB)] for c in range(NCHUNK)]

            def dma_chunk(c):
                for o in range(KSUB):
                    nc.sync.dma_start(xt[c][o][:], xt_r[:, o, c * CH:(c + 1) * CH])

            # chunk 0 first on the DMA engines, then the small consts, then
            # the rest -- so the first projection starts ~2us earlier.
            dma_chunk(0)
            nc.sync.dma_start(wq[:], wq_r[:])
            nc.sync.dma_start(bt[:], bt_d[:])
            # Causal diag-block masks generated on the idle GpSimd engine:
            # mask[p, j, f] = 1.0 if j*128 + p <= f else 0.0
            nc.gpsimd.memset(maskt[:], 0.0)
            for j in range(NBLK):
                nc.gpsimd.affine_select(
                    out=maskt[:, j, :],
                    in_=maskt[:, j, :],
                    compare_op=mybir.AluOpType.is_gt,
                    fill=1.0,
                    base=j * P,
                    pattern=[[-1, CH]],
                    channel_multiplier=1,
                )
            for c in range(1, NCHUNK):
                dma_chunk(c)

            # Primer ops: make DVE/ACT observe the const-DMA queue semaphores
            # early so steady-state instructions carry only engine-sem waits.
            prime = consts.tile([P, 1], f32)
            # Exp on the (gpsimd-generated, DMA-free) identity pulls the ACT
            # exp table load (~2.7us) to t~0, off the first-scores critical
            # path; the second primer makes ACT observe the bt DMA queue.
            nc.scalar.activation(prime[:H, :], ident[:, 0:1], AF.Exp)
            nc.vector.tensor_copy(out=prime[:], in_=biases[:, 0:1])
            nc.vector.tensor_copy(out=prime[:], in_=maskt[:, 0, 0:1])
            nc.vector.tensor_copy(out=prime[:], in_=theta[:, 0:1])
            nc.scalar.activation(prime[:], theta[:, 1:2], AF.Exp)

            # kt/qd live on partitions 64:128 so the h=64-contraction scores
            # matmul has consistently-based operands (array rows 64:127).
            # vkt[c]: rows 0:64 = vT chunk, rows 64:128 = kT chunk -- one
            # bias-add writes both halves from the packed [Wv|Wk] PSUM.
            vkt = [data.tile([P, CH], bf16, tag=f"vkt{c}", name=f"vkt{c}") for c in range(NCHUNK)]
            vext = [data.tile([P, NBLK, P], bf16, tag=f"vx{c}", name=f"vx{c}") for c in range(NCHUNK)]
            qd = [data.tile([P, CH], bf16, tag=f"qd{s}", name=f"qd{s}") for s in range(2)]
            outsb = [data.tile([H + 1, CH], f32, tag=f"outsb{s}", name=f"outsb{s}")
                     for s in range(2)]

            po = [popool.tile([P, CH], f32, tag=f"po{s}", name=f"po{s}") for s in range(2)]
            av_count = [0, 0]
            AV_TOTAL = [NBLK + 4, 2 * NBLK + 8]  # 8, 16

            def proj_chunk(c):
                # [vT; kT] chunk = [Wv|Wk]^T @ xT_chunk, accumulated over KSUB
                ps1 = pspool.tile([P, CH], f32, tag="ps", name="ps")
                for o in range(KSUB):
                    nc.tensor.matmul(
                        ps1[:],
                        wvk[:, o, :],
                        xt[c][o][:],
                        start=(o == 0),
                        stop=(o == KSUB - 1),
                    )
                nc.vector.tensor_scalar_add(vkt[c][:], ps1[:], biases[:, 0:1])
                if c < 2:
                    # qT for own rows; output to PSUM partitions 64:128 via
                    # col-group tile_position so no cross-partition copies.
                    ps2 = pspool.tile([P, CH], f32, tag="ps", name="ps")
                    for o in range(KSUB):
                        nc.tensor.matmul(
                            ps2[H:, :],
                            wq[:, o, :],
                            xt[c][o][:],
                            start=(o == 0),
                            stop=(o == KSUB - 1),
                            tile_position=(0, 64),
                        )
                    nc.vector.tensor_scalar_add(qd[c][H:, :], ps2[H:, :], biases[H:, 1:2])
                # v blocks: PE-transpose vT -> 4x [128, 64] into one PSUM
                # tile, one strided copy-back, ones col appended.  Other-half
                # gating happens via the exp bias, not here.
                nc.vector.memset(vext[c][:, :, H:], 0.0)
                nc.vector.memset(vext[c][:, :, H:H + 1], 1.0)
                pst = pstpool.tile([P, NBLK, H], bf16, tag="pst", name="pst")
                for b in range(NBLK):
                    nc.tensor.transpose(pst[:, b, :], vkt[c][:H, b * P:(b + 1) * P], ident[:])
                nc.vector.tensor_copy(out=vext[c][:, :, :H], in_=pst[:])

            def attn_pair(s, kc):
                for h in range(2):
                    k = kc + h
                    ps = pspool.tile([P, CH], f32, tag="ps", name="ps")
                    nc.tensor.matmul(
                        ps[:],
                        vkt[k // NBLK][H:, (k % NBLK) * P:(k % NBLK + 1) * P],
                        qd[s][H:, :],
                    )
                    pt = ptpool.tile([P, CH], bf16, tag="pt", name="pt")
                    # Cross-chunk blocks (k >= 8): per-core additive bias
                    # before exp -- 0.0 where allowed, -40 where fully masked.
                    if k < 8:
                        bc = 0
                    elif s == 0:
                        bc = 1
                    else:
                        bc = 2 if k < 12 else 3
                    bias = theta[:, bc:bc + 1]
                    nc.scalar.activation(pt[:], ps[:], AF.Exp, bias=bias)
                    j = k - NBLK * s
                    if k < 8 and 0 <= j < NBLK:
                        nc.vector.tensor_mul(pt[:], pt[:], maskt[:, j, :])
                    i = av_count[s]
                    nc.tensor.matmul(
                        po[s][:],
                        vext[k // NBLK][:, k % NBLK, :],
                        pt[:],
                        start=(i == 0),
                        stop=(i == AV_TOTAL[s] - 1),
                    )
                    av_count[s] = i + 1

            def flush_slot(s):
                nc.vector.tensor_copy(out=outsb[s][:], in_=po[s][:H + 1, :])
                nc.sync.dma_start(out_d[:, s * CH:(s + 1) * CH], outsb[s][:])

            proj_chunk(0)
            for kc in (0, 2):
                attn_pair(0, kc)
            proj_chunk(1)
            for kc in (0, 2, 4, 6):
                attn_pair(1, kc)
            # Final stages interleaved: proj3 and slot-1's rest1 blocks mix
            # into the rest0 stream so ACT never runs dry and the wind-down
            # after the last scores matmul is minimal.
            proj_chunk(2)
            attn_pair(0, 8)
            attn_pair(1, 8)
            proj_chunk(3)
            attn_pair(0, 10)
            flush_slot(0)
            attn_pair(1, 12)
            attn_pair(1, 10)
            attn_pair(1, 14)
            flush_slot(1)

    nc.compile()
    return nc


def make_in_maps(x, Wq, bq, Wk, bk, Wv, bv):
    import ml_dtypes
    bf16 = ml_dtypes.bfloat16
    x = np.asarray(x, dtype=np.float32)
    scale = 1.0 / np.sqrt(np.float32(H))
    wvk_h = np.concatenate(
        [np.asarray(Wv, np.float32), np.asarray(Wk, np.float32)], axis=1
    ).astype(bf16).reshape(KSUB, P, P)
    wq_h = (np.asarray(Wq, np.float32) * scale).astype(bf16).reshape(KSUB, P, H)
    # [P, KSUB*P] and [P, KSUB*H], partition-major, concatenated
    wall = np.ascontiguousarray(np.concatenate(
        [wvk_h.transpose(1, 0, 2).reshape(P, KSUB * P),
         wq_h.transpose(1, 0, 2).reshape(P, KSUB * H)], axis=1))
    bias = np.zeros((P, 2), np.float32)
    bias[:H, 0] = np.asarray(bv, np.float32)
    bias[H:, 0] = np.asarray(bk, np.float32)
    bias[H:, 1] = np.asarray(bq, np.float32) * scale
    in_maps = []
    for c in range(8):
        b, g = c // 2, c % 2
        # core g=0 owns original q-chunks (0, 3); g=1 owns (1, 2) -- balanced
        # causal work.  Permuted row order: [A; B; rest0; rest1].
        A, Bc, r0, r1 = ((0, 3, 1, 2) if g == 0 else (1, 2, 0, 3))
        perm = np.concatenate([np.arange(cc * CH, (cc + 1) * CH) for cc in (A, Bc, r0, r1)])
        xT = np.ascontiguousarray(x[b][perm].T.astype(bf16))
        bt = np.zeros((P, 6), np.float32)
        bt[:, 0:2] = bias
        # col3: slot0 vs rest0; col4: slot1 vs rest0; col5: slot1 vs rest1
        bt[:, 3] = 0.0 if r0 < A else -40.0
        bt[:, 4] = 0.0 if r0 < Bc else -40.0
        bt[:, 5] = 0.0 if r1 < Bc else -40.0
        in_maps.append({"xt": xT, "wall": wall, "bt": bt})
    return in_maps


def gather(results):
    out = np.zeros((B, S, H), np.float32)
    for c in range(8):
        b, g = c // 2, c % 2
        A, Bc = (0, 3) if g == 0 else (1, 2)
        r = results[c]["out"]  # [65, 1024]
        o = (r[:H] / r[H:H + 1]).T
        out[b, A * CH:(A + 1) * CH] = o[:CH]
        out[b, Bc * CH:(Bc + 1) * CH] = o[CH:]
    return out


def kernel(x, Wq, bq, Wk, bk, Wv, bv):
    global _NC
    from concourse.bass_utils import run_bass_kernel_spmd

    if _NC is None:
        _NC = build_bass()
    in_maps = make_in_maps(x, Wq, bq, Wk, bk, Wv, bv)
    res = run_bass_kernel_spmd(_NC, in_maps, core_ids=list(range(8)), trace=TRACE)
    LAST["res"] = res
    return gather(res.results)



# revision 20
# speedup vs baseline: 1.4357x; 1.4357x over previous
"""Causal single-head attention (b=4, s=2048, d=1024, h=64) on 8 TRN2 cores.

Sharding: core c -> (batch b = c//2, g = c%2); the core owns the balanced
q-chunk pair A,B = (0,3) if g==0 else (1,2) (512 queries each).  Permuted row
order [A; B; rest0; rest1] makes one uniform SPMD program valid for all cores;
fully-masked cross-chunk visits are gated by a per-core additive exp bias in
{0, -40} from the packed bt input.

Cost-model-shaped design (TimelineSim):
  - x and all weights are fp8 (e4m3, weights pre-scaled x16 on the host) and
    the V/K/Q projections run as DoubleRow matmuls: each instruction
    contracts 256 of d per step at 0.5 cycles/row -> 4 instructions per
    512-seq chunk instead of 8, and the x DMA halves.
  - Scores stay bf16 (q/k reconstructed at true scale by the PSUM
    evacuation (ps/16 + bias)); the 1/sqrt(h) lands in the exp scale.
  - The causal triangle of diagonal blocks is ADDED into the scores PSUM as
    an fp8-DoubleRow matmul (identity @ host-baked staircase of -160), so
    exp sees pre-masked scores and DVE never touches the attention path.
  - Scores for a visit are computed into a [128, 2, 512] two-bank PSUM pair
    and exp'd by ONE ACT instruction into a [128, 2, 512] fp8 pt pair; the
    AV matmul consumes the pair as a DoubleRow contraction over 256 keys,
    emitting [o_unnorm; denom] via a ones-column in the padded [128,2,80]
    V tile.  Diagonal pair-1 (keys 256:512) is exp'd only on queries
    256:512 into a pre-zeroed pt tile.
  - PSUM->SBUF evacuations are split across DVE (vkt, final flush) and
    GpSimd (qd, flush0); outputs DMA per-slot from SBUF.

Requires bacc.Bacc + nc.compile().
"""

import numpy as np

B, S, D, H = 4, 2048, 1024, 64
P = 128
HALF = S // 2          # 1024 queries per core
CH = 512               # free-dim chunk (PSUM bank = 512 fp32)
KT = 4                 # DoubleRow contraction tiles (256 each) over d=1024
NCHUNK = S // CH       # 4 sequence chunks
VW = 80                # padded AV weight cols: [v(64) | ones(1) | 0...] %16
WS = 16.0              # host weight prescale (exact power of 2)

_NC = None
TRACE = False
LAST = {}


def build_bass():
    import concourse.bass as bass  # noqa: F401
    import concourse.mybir as mybir
    import concourse.tile as tile
    from concourse import bacc
    from concourse.masks import make_identity

    f32 = mybir.dt.float32
    bf16 = mybir.dt.bfloat16
    fp8 = mybir.dt.float8e4
    AF = mybir.ActivationFunctionType
    DR = mybir.MatmulPerfMode.DoubleRow

    nc = bacc.Bacc()
    # x pre-permuted/transposed/fp8 on host: xt[p, k, t, s] = xT[k*256+t*128+p, s]
    xt_d = nc.dram_tensor("xt", [P, 2 * KT, S], fp8, kind="ExternalInput")
    # [wvk (4*2*128) | wq (4*2*64)] fp8, x16-prescaled
    wall_d = nc.dram_tensor("wall", [P, 2 * KT * (P + H)], fp8, kind="ExternalInput")
    # [stair (4*512) | identz (2*128) | identz2 (2*128)] fp8
    msk_d = nc.dram_tensor("msk", [P, 4 * CH + 4 * P], fp8, kind="ExternalInput")
    bt_d = nc.dram_tensor("bt", [P, 8], f32, kind="ExternalInput")
    # bf16 early-rows patch operands: x rows 0:64 and [Wq|Wk|Wv] unscaled
    xp_d = nc.dram_tensor("xp", [P, 8, H], bf16, kind="ExternalInput")
    wp_d = nc.dram_tensor("wp", [P, 8, 3 * H], bf16, kind="ExternalInput")
    out_d = nc.dram_tensor("out", [H + 1, HALF + H], f32, kind="ExternalOutput")

    xt_r = xt_d.rearrange("p (k t) s -> p k t s", k=KT)
    wvk_r = wall_d[:, :2 * KT * P].rearrange("p (k t m) -> p k t m", k=KT, t=2)
    wq_r = wall_d[:, 2 * KT * P:].rearrange("p (k t m) -> p k t m", k=KT, t=2)
    msk_r = msk_d.rearrange("p (j q) -> p j q", q=CH)  # j: 0-3 stair, 4 idz, 4.5 idz2

    with tile.TileContext(nc) as tc:
        with (
            tc.tile_pool(name="consts", bufs=1) as consts,
            tc.tile_pool(name="data", bufs=1) as data,
            tc.tile_pool(name="pp", bufs=2, space="PSUM") as pairpool,
            tc.tile_pool(name="pj", bufs=2, space="PSUM") as projpool,
            tc.tile_pool(name="po", bufs=1, space="PSUM") as popool,
            tc.tile_pool(name="pt", bufs=3) as ptpool,
        ):
            wvk = consts.tile([P, KT, 2, P], fp8)
            wq = consts.tile([P, KT, 2, H], fp8)
            stair = consts.tile([P, 4, CH], fp8)
            idz = consts.tile([P, 2, 2, P], fp8)   # [pair-sel][t][col]
            bt = consts.tile([P, 8], f32)
            identT = consts.tile([H, H], bf16)
            xp = consts.tile([P, 8, H], bf16)
            wp = consts.tile([P, 8, 3 * H], bf16)

            xt = [data.tile([P, KT, 2, CH], fp8, tag=f"xt{c}", name=f"xt{c}")
                  for c in range(NCHUNK)]

            # DMA order tuned for the front-end critical path.
            nc.sync.dma_start(wvk[:], wvk_r[:])
            nc.sync.dma_start(wq[:], wq_r[:])
            nc.sync.dma_start(xt[0][:], xt_r[:, :, :, 0:CH])
            nc.sync.dma_start(bt[:], bt_d[:])
            nc.sync.dma_start(stair[:], msk_r[:, 0:4, :])
            nc.sync.dma_start(
                idz[:],
                msk_d[:, 4 * CH:].rearrange("p (a t m) -> p a t m", t=2, m=P),
            )
            for c in range(1, NCHUNK):
                nc.sync.dma_start(xt[c][:], xt_r[:, :, :, c * CH:(c + 1) * CH])
            nc.sync.dma_start(wp[:], wp_d[:])
            nc.sync.dma_start(xp[:], xp_d[:])

            make_identity(nc, identT[:])

            vkt = [data.tile([P, CH], bf16, tag=f"vkt{c}", name=f"vkt{c}") for c in range(NCHUNK)]
            vext = [data.tile([P, 2, 2, VW], fp8, tag=f"vx{c}", name=f"vx{c}") for c in range(NCHUNK)]
            qd = [data.tile([P, CH], bf16, tag=f"qd{s}", name=f"qd{s}") for s in range(2)]
            outsb = [data.tile([H + 1, CH + (H if s else 0)], f32,
                               tag=f"outsb{s}", name=f"outsb{s}")
                     for s in range(2)]
            qp = data.tile([H, H], bf16, tag="qp", name="qp")
            kp = data.tile([H, H], bf16, tag="kp", name="kp")
            vsp = data.tile([H, H + 1], bf16, tag="vsp", name="vsp")
            ptp = data.tile([H, H], bf16, tag="ptp", name="ptp")
            # pre-zeroed pt tiles for the trimmed diagonal pair-1 exps
            pt_trim = [data.tile([P, 2, CH], fp8, tag=f"ptt{s}", name=f"ptt{s}") for s in range(2)]
            for s in range(2):
                z16 = pt_trim[s][:, :, 0:CH // 2].bitcast(mybir.dt.uint16)
                nc.vector.memset(z16, 0)

            # Primer ops: ACT exp-table load at t~0 (identT is DMA-free), and
            # early queue-semaphore observation for ACT/DVE/Pool.
            prime = consts.tile([P, 1], f32)
            nc.scalar.activation(prime[:H, :], identT[:, 0:1], AF.Exp)
            nc.vector.tensor_copy(out=prime[:], in_=bt[:, 0:1])
            nc.gpsimd.tensor_copy(out=prime[:], in_=bt[:, 1:2])
            nc.scalar.activation(prime[:], bt[:, 2:3], AF.Exp)
            nc.vector.tensor_copy(out=prime[:H], in_=stair[:H, 0, 0:1])

            po = [popool.tile([VW, CH], f32, tag=f"po{s}", name=f"po{s}") for s in range(2)]
            av_n = [0, 0]
            AV_TOTAL = [4, 8]  # pairs per slot

            def proj_chunk(c):
                ps1 = projpool.tile([P, CH], f32, tag="ps", name=f"ps1_{c}")
                for k in range(KT):
                    nc.tensor.matmul(
                        ps1[:], wvk[:, k], xt[c][:, k],
                        start=(k == 0), stop=(k == KT - 1), perf_mode=DR,
                    )
                nc.vector.tensor_scalar(
                    out=vkt[c][:], in0=ps1[:],
                    scalar1=1.0 / WS, scalar2=bt[:, 0:1],
                    op0=mybir.AluOpType.mult, op1=mybir.AluOpType.add,
                )
                if c < 2:
                    ps2 = projpool.tile([P, CH], f32, tag="ps", name=f"ps2_{c}")
                    for k in range(KT):
                        nc.tensor.matmul(
                            ps2[:H, :], wq[:, k], xt[c][:, k],
                            start=(k == 0), stop=(k == KT - 1), perf_mode=DR,
                        )
                    nc.vector.tensor_scalar(
                        out=qd[c][H:, :], in0=ps2[:H, :],
                        scalar1=1.0 / WS, scalar2=bt[H:, 1:2],
                        op0=mybir.AluOpType.mult, op1=mybir.AluOpType.add,
                    )
                pst = projpool.tile([P, 2, 2, H], bf16, tag="ps", name=f"pst{c}")
                for j in range(4):
                    nc.tensor.transpose(
                        pst[:, j // 2, j % 2, :],
                        vkt[c][:H, j * P:(j + 1) * P], identT[:],
                    )
                nc.gpsimd.memset(vext[c][:, :, :, H:VW], 0.0)
                nc.gpsimd.memset(vext[c][:, :, :, H:H + 1], 1.0)
                nc.vector.tensor_copy(out=vext[c][:, :, :, 0:H], in_=pst[:])

            def visit(s, pc, bc):
                """One (slot, perm-chunk) visit: 2 score-pairs -> 2 exps -> 2 AV."""
                diag = (pc == s)
                for p in range(2):
                    trim = diag and p == 1
                    lo = CH // 2 if trim else 0
                    ps = pairpool.tile([P, 2, CH], f32, tag="pp", name=f"pp{s}_{pc}_{p}")
                    for t in range(2):
                        j = 2 * p + t
                        if diag:
                            # additive -160 staircase via fp8-DR identity
                            # matmul; j=3 uses the t=1-active identity so the
                            # stair slice stays in-bounds.
                            zsel, jlo = (0, j) if j < 3 else (1, 2)
                            nc.tensor.matmul(
                                ps[:, t, lo:CH], idz[:, zsel],
                                stair[:, jlo:jlo + 2, lo:CH],
                                start=True, stop=False, perf_mode=DR,
                            )
                        nc.tensor.matmul(
                            ps[:, t, lo:CH],
                            vkt[pc][H:, j * P:(j + 1) * P],
                            qd[s][H:, lo:CH],
                            start=not diag, stop=True,
                        )
                    pt = (pt_trim[s] if trim else
                          ptpool.tile([P, 2, CH], fp8, tag="pt", name=f"pt{s}_{pc}_{p}"))
                    nc.scalar.activation(
                        pt[:, :, lo:CH], ps[:, :, lo:CH], AF.Exp,
                        bias=bt[:, bc:bc + 1], scale=0.125,
                    )
                    i = av_n[s]
                    nc.tensor.matmul(
                        po[s][:], vext[pc][:, p], pt[:],
                        start=(i == 0), stop=(i == AV_TOTAL[s] - 1), perf_mode=DR,
                    )
                    av_n[s] = i + 1

            def patch():
                """bf16 recompute of rows 0:64 x keys 0:64; host uses it for
                the cores owning chunk 0."""
                A = mybir.AluOpType
                psqk = projpool.tile([P, H], f32, tag="ps", name="psqk")
                for o in range(8):
                    nc.tensor.matmul(psqk[:], wp[:, o, 0:2 * H], xp[:, o, :],
                                     start=(o == 0), stop=(o == 7))
                psv = projpool.tile([H, H], f32, tag="ps", name="psv")
                for o in range(8):
                    nc.tensor.matmul(psv[:], wp[:, o, 2 * H:], xp[:, o, :],
                                     start=(o == 0), stop=(o == 7))
                nc.vector.tensor_scalar_add(qp[:], psqk[:H, :], bt[:H, 6:7])
                nc.vector.tensor_scalar_add(kp[:], psqk[H:, :], bt[H:, 6:7])
                vtp = data.tile([H, H], bf16, tag="vtp", name="vtp")
                nc.vector.tensor_scalar_add(vtp[:], psv[:], bt[:H, 7:8])
                psts = projpool.tile([H, H], bf16, tag="ps", name="psts")
                nc.tensor.transpose(psts[:], vtp[:], identT[:])
                nc.vector.memset(vsp[:, H:H + 1], 1.0)
                nc.vector.tensor_copy(out=vsp[:, 0:H], in_=psts[:])
                pss = projpool.tile([H, H], f32, tag="ps", name="pss")
                nc.tensor.matmul(pss[:], idz[:H, 0, 0, :H], stair[:H, 0, :H],
                                 start=True, stop=False)
                nc.tensor.matmul(pss[:], kp[:], qp[:], start=False, stop=True)
                nc.scalar.activation(ptp[:], pss[:], AF.Exp, bias=0.0, scale=0.125)
                pop = projpool.tile([H + 1, H], f32, tag="ps", name="pop")
                nc.tensor.matmul(pop[:], vsp[:], ptp[:], start=True, stop=True)
                nc.vector.tensor_copy(out=outsb[1][:, CH:], in_=pop[:])

            def flush(s):
                nc.vector.tensor_copy(out=outsb[s][:, 0:CH], in_=po[s][:H + 1, :])
                nc.sync.dma_start(
                    out_d[:, s * CH:(s + 1) * CH + (H if s else 0)], outsb[s][:])

            proj_chunk(0)
            visit(0, 0, 2)          # slot0 diag
            proj_chunk(1)
            visit(1, 1, 2)          # slot1 diag
            visit(1, 0, 2)          # slot1 x chunk A (always causal-allowed)
            proj_chunk(2)
            visit(0, 2, 3)          # slot0 x rest0 (bias-gated)
            flush(0)
            visit(1, 2, 4)          # slot1 x rest0 (bias-gated)
            proj_chunk(3)
            patch()
            visit(1, 3, 5)          # slot1 x rest1 (bias-gated)
            flush(1)

    nc.compile()
    return nc


def make_in_maps(x, Wq, bq, Wk, bk, Wv, bv):
    import ml_dtypes
    e4 = ml_dtypes.float8_e4m3
    bf = ml_dtypes.bfloat16
    x = np.asarray(x, dtype=np.float32)

    def dr_pack(w):  # [1024, M] -> [128, KT, 2, M]
        m = w.shape[1]
        return np.ascontiguousarray(
            w.reshape(KT, 2, P, m).transpose(2, 0, 1, 3)).astype(e4)

    wvk8 = dr_pack(np.concatenate(
        [np.asarray(Wv, np.float32), np.asarray(Wk, np.float32)], axis=1) * WS)
    wq8 = dr_pack(np.asarray(Wq, np.float32) * WS)
    wall = np.ascontiguousarray(np.concatenate(
        [wvk8.reshape(P, KT * 2 * P), wq8.reshape(P, KT * 2 * H)], axis=1))

    # stair[p, j, q] = -160 if j*128 + p > q else 0
    pidx = np.arange(P)[:, None, None]
    jidx = np.arange(4)[None, :, None]
    qidx = np.arange(CH)[None, None, :]
    stairs = np.where(jidx * P + pidx > qidx, np.float32(-160.0), np.float32(0.0))
    ident = np.eye(P, dtype=np.float32)
    idz = np.zeros((P, 2, 2, P), np.float32)
    idz[:, 0, 0, :] = ident
    idz[:, 1, 1, :] = ident
    msk = np.ascontiguousarray(np.concatenate(
        [stairs.reshape(P, 4 * CH), idz.reshape(P, 4 * P)], axis=1)).astype(e4)

    bias = np.zeros((P, 2), np.float32)
    bias[:H, 0] = np.asarray(bv, np.float32)
    bias[H:, 0] = np.asarray(bk, np.float32)
    bias[H:, 1] = np.asarray(bq, np.float32)

    # bf16 patch weights [Wq|Wk|Wv] unscaled, subtiled [128, 8, 192]
    wp = np.ascontiguousarray(
        np.concatenate([np.asarray(Wq, np.float32), np.asarray(Wk, np.float32),
                        np.asarray(Wv, np.float32)], axis=1)
        .reshape(8, P, 3 * H).transpose(1, 0, 2)).astype(bf)

    in_maps = []
    for c in range(8):
        b, g = c // 2, c % 2
        A, Bc, r0, r1 = ((0, 3, 1, 2) if g == 0 else (1, 2, 0, 3))
        perm = np.concatenate([np.arange(cc * CH, (cc + 1) * CH) for cc in (A, Bc, r0, r1)])
        xT = x[b][perm].T  # [1024, 2048]
        xt8 = np.ascontiguousarray(
            xT.reshape(KT, 2, P, S).transpose(2, 0, 1, 3).reshape(P, 2 * KT, S)
        ).astype(e4)
        xp = np.ascontiguousarray(
            x[b][:H].T.reshape(8, P, H).transpose(1, 0, 2)).astype(bf)
        bt = np.zeros((P, 8), np.float32)
        bt[:, 0:2] = bias
        # col2: always-allowed (diag + slot1 x chunk A); 3,4,5: gated rests
        bt[:, 3] = 0.0 if r0 < A else -40.0
        bt[:, 4] = 0.0 if r0 < Bc else -40.0
        bt[:, 5] = 0.0 if r1 < Bc else -40.0
        bt[:H, 6] = np.asarray(bq, np.float32)
        bt[H:, 6] = np.asarray(bk, np.float32)
        bt[:H, 7] = np.asarray(bv, np.float32)
        in_maps.append({"xt": xt8, "wall": wall, "msk": msk, "bt": bt,
                        "xp": xp, "wp": wp})
    return in_maps


def gather(results):
    out = np.zeros((B, S, H), np.float32)
    for c in range(8):
        b, g = c // 2, c % 2
        A, Bc = (0, 3) if g == 0 else (1, 2)
        r = results[c]["out"]  # [65, 1024 + 64]
        o = (r[:H, :HALF] / r[H:H + 1, :HALF]).T
        out[b, A * CH:(A + 1) * CH] = o[:CH]
        out[b, Bc * CH:(Bc + 1) * CH] = o[CH:]
        if A == 0:
            p = r[:, HALF:]  # bf16 early-rows patch (rows 0:64, device)
            out[b, :H] = (p[:H] / p[H:H + 1]).T
    return out


def kernel(x, Wq, bq, Wk, bk, Wv, bv):
    global _NC
    from concourse.bass_utils import run_bass_kernel_spmd

    if _NC is None:
        _NC = build_bass()
    in_maps = make_in_maps(x, Wq, bq, Wk, bk, Wv, bv)
    res = run_bass_kernel_spmd(_NC, in_maps, core_ids=list(range(8)), trace=TRACE)
    LAST["res"] = res
    return gather(res.results)
